# revision 14
# baseline (speedup 1.0000x reference)
"""Dark channel prior loss on 8 trn2 NeuronCores.

Reference computes: reflect-pad H/W by 7, min over (C, H, W) per image,
mean over batch. Reflect padding only duplicates interior values, so it
cannot change a min — the loss is exactly mean_b(min_chw(x[b])).

Data-parallel: 4 images (12 MiB) per core, streamed as 18 chunks of
[128, .] f32 tapering 2048 -> 1536 -> 1024 columns (scheduled so
VectorE's pipelined per-chunk reduces drain within ~one small chunk of
the stream's end — see the DP note at CHUNK_SIZES).

Loads ride the SP-engine HWDGE ring (nc.sync.dma_start), not SWDGE:
SWDGE keeps its descriptor rings in SBUF partitions whose AXI ports
also serve SDMA engines 7/15, and the resulting fetch contention ran
engine 15 at ~22.5 GB/s vs ~25.7 for the rest — every chunk's 16-inc
completion waited on it, pushing the last reduce ~7 us past the end of
the stream. HWDGE descriptors are RTL-generated (no SBUF ring), the
engines stay uniform, and dispatch skips the ~650 ns/DMA Q7 emission.

Raw bacc kernel (no TileContext): the load DMAs are spliced between
SP's init-barrier entry (so the barrier's gather inc — and DVE's
release — isn't held hostage by the ~0.6-1.5 us/DMA dispatch train)
and its barrier-exit wait (so the stream starts ~0.5 us before Pool
releases the barrier). VectorE min-reduces each chunk to one column of
a [128, 18] partial as its chunk's completion sem hits 16 (one sem per
DMA — a shared cumulative counter is unsound across interleaved
per-engine increments). The partial is DMA'd out on Pool/SWDGE with no
completion wait (the runtime's end-of-program Pool DRAIN blocks on
SWDGE queue completion — measured) while one range-clear resets the
sems for repeat executions. The host finishes min-over-partitions/
chunks and the batch mean (tiny).

History (max-core / mean, us): SWDGE loads 50.5/45.6 (random
~22.4 GB/s straggler SDMA engine on 1-3 cores serializes the tail —
it moves between runs/cores/engines and hits HWDGE too; the kernel
just minimizes the tail it can gate). HWDGE loads, 7x3072+taper:
48.1/44.8 (tail reduces stack serially behind big-chunk reduces).
24x1024 + SP out-DMA: 62.1/58.1 (loads-before-barrier-entry held DVE's
release hostage for the whole 24-DMA dispatch train; out-on-SP put the
out receipt on the critical path).
"""

import numpy as np

import concourse.bass as bass  # noqa: F401
from concourse import bacc, mybir
from concourse.bass_utils import run_bass_kernel_spmd


def _install_ntff_hook():
    """This image's antenv lacks axon_hooks, so a traced run (trace=True or
    BASS_TRACE=1) would crash inside run_bass_kernel_spmd on the import.
    Synthesize the module around trn_boot's ctypes NTFF hook; degrade
    silently if any piece is missing."""
    import sys
    import types

    if "antenv.axon_hooks" in sys.modules:
        return
    try:
        sys.path.insert(0, "/root/.axon_site")
        from trn_agent_boot.trn_boot import _ntff_profile_via_ctypes

        hook = _ntff_profile_via_ctypes("/opt/axon/libaxon_pjrt.so")
        mod = types.ModuleType("antenv.axon_hooks")
        mod._hook = hook
        mod.get_axon_ntff_profile_hook = lambda: mod._hook
        mod.set_axon_ntff_profile_hook = lambda h: setattr(mod, "_hook", h)
        sys.modules["antenv.axon_hooks"] = mod
    except Exception:
        pass


_install_ntff_hook()

N_CORES = 8
B = 32
PER_CORE = B // N_CORES  # 4 images per core
P = 128
F = 3 * 512 * 512 // P  # 6144 elements per partition per image
TOTAL = PER_CORE * F  # 24576 columns of [128, .] per core

# Chunk column-widths: 3072 -> 512 taper, DP-optimized for the
# STRAGGLER core. The last reduce ends at max_c(land_c +
# suffix_reduce_work_c); DVE reduces at ~1.10 ns/col + 90 ns/op, the
# stream delivers at ~(1.18 + 72/w) ns/col (per-row descriptors are
# w*4 bytes, so narrow chunks pay more per-packet overhead). Nearly
# every launch has 1-2 cores where ONE SDMA engine runs ~13-18% slow
# (worse for small packets; engine/core varies per run — seen on SWDGE
# and HWDGE alike), and the harness takes max over cores, so the DP
# objective dilates stream time by (1.12 + 80/w) per chunk: big early
# chunks minimize the straggler's per-packet overhead, the small tail
# keeps the post-stream reduce drain short on every core. Boundaries
# stay inside images (F=6144 aligned), so each partial column belongs
# to one image.
CHUNK_SIZES = [3072] * 4 + [2048] * 3 + [1536] * 2 + [1024, 768, 768, 512]
assert sum(CHUNK_SIZES) == TOTAL
CHUNK_STARTS = [sum(CHUNK_SIZES[:i]) for i in range(len(CHUNK_SIZES))]
for _s, _w in zip(CHUNK_STARTS, CHUNK_SIZES):
    assert _s // F == (_s + _w - 1) // F
NCHUNK = len(CHUNK_SIZES)
COL_IMG = [s // F for s in CHUNK_STARTS]  # partial column -> image index

_nc_cache = None


def _build_nc(optimize: bool = True):
    nc = bacc.Bacc(trn_type="TRN2", debug=False, num_devices=N_CORES)
    x = nc.dram_tensor("x", [PER_CORE, P, F], mybir.dt.float32, kind="ExternalInput")
    out = nc.dram_tensor("out", [P, NCHUNK], mybir.dt.float32, kind="ExternalOutput")
    x_ap = x.ap()
    out_ap = out.ap()

    # One completion sem per chunk DMA, waited to exactly 16 (one inc per
    # SDMA engine). A single cumulative counter would be unsound: engine k
    # of a LATER chunk can increment before engine j of chunk c finishes,
    # satisfying a >=16*(c+1) wait while chunk c's rows are still in
    # flight (observed as sporadic wrong partials).
    chunk_sems = [nc.alloc_semaphore(f"dma_done_{c}") for c in range(NCHUNK)]
    red_sem = nc.alloc_semaphore("red_done")
    out_sem = nc.alloc_semaphore("out_done")
    buf = nc.alloc_sbuf_tensor("buf", [P, TOTAL], mybir.dt.float32)
    partial = nc.alloc_sbuf_tensor("partial", [P, NCHUNK], mybir.dt.float32)

    # Loads alternate between the two HWDGE rings (SP: qSPDynamicHW,
    # ACT: qActDynamicHW). Each SDMA engine round-robins between rings
    # at packet granularity, so with both rings fed the per-engine
    # descriptor pipeline stays deeper; it also halves each sequencer's
    # ~0.6-1.5 us/DMA dispatch train. Chunk completion sems are
    # per-chunk, so ring interleaving only jitters land order slightly.
    load_by_eng = {id(nc.sync): [], id(nc.scalar): []}
    for c, (s, w) in enumerate(zip(CHUNK_STARTS, CHUNK_SIZES)):
        b, off = s // F, s % F  # chunks never straddle an image boundary
        eng = (nc.sync, nc.scalar)[c % 2]
        bi = eng.dma_start(
            buf.ap()[:, s : s + w], x_ap[b][:, off : off + w]
        ).then_inc(chunk_sems[c], 16)
        load_by_eng[id(eng)].append(bi.ins)
    for c, (s, w) in enumerate(zip(CHUNK_STARTS, CHUNK_SIZES)):
        red = nc.vector.tensor_reduce(
            out=partial.ap()[:, c : c + 1],
            in_=buf.ap()[:, s : s + w],
            axis=mybir.AxisListType.X,
            op=mybir.AluOpType.min,
        )._wait_ge(chunk_sems[c], 16)
        if c == NCHUNK - 1:
            red.then_inc(red_sem)
    out_bi = nc.gpsimd.dma_start(out_ap[:], partial.ap())._wait_ge(
        red_sem, 1
    ).then_inc(out_sem, 16)
    # The out-DMA stays on Pool/SWDGE: routing it via SP's HWDGE ring and
    # gating the clear on out_sem>=16 was measured +1.1 us — it saves the
    # ~0.67 us Q7 emission but puts the out's HBM write receipt on the
    # critical path before the done protocol. Here the range-clear (which
    # resets kernel sems so a repeat execution of the same NEFF starts
    # clean) runs right after the out-DMA's Q7 emission and overlaps its
    # flight; chunk/red sems are final-valued once red_sem fired. Nothing
    # waits on out_sem (the DMA lowering just needs an update target):
    # the runtime's end-of-program Pool DRAIN blocks until the SWDGE
    # queue has fully completed (measured), which is what guarantees the
    # output landed before the NEFF execution retires.
    assert out_sem.num == chunk_sems[0].num + NCHUNK + 1
    nc.gpsimd.sem_clear(range(chunk_sems[0].num, out_sem.num + 1))

    if optimize:
        # Hoist the load DMAs to right after SP's init-barrier ENTRY (its
        # InstDrain, which also incs the barrier gather sem) and before
        # its barrier-exit wait. Placement matters on both sides:
        #  - After the entry inc, NOT before it: SP dispatches each
        #    DMACopy in ~0.6-1.5 us (qSPDynamicHW backpressure included),
        #    so loads placed before the barrier entry hold the gather inc
        #    — and with it DVE's release — hostage for the whole dispatch
        #    train (measured +13 us: every reduce then serialized after
        #    the stream).
        #  - Before the exit wait, NOT after it: the barrier release only
        #    comes once Pool's memsets/drain finish (~6 us), while SP is
        #    ready at ~5.4 us — dispatching the first load immediately
        #    starts the HBM stream ~0.5 us earlier.
        # The drain itself stays real: SP has no outstanding DMAs at the
        # barrier (the loads now follow it), so it completes in ns. The
        # loads still execute after the NRT pseudo-barrier that fences
        # the init-time sem_clear, so the chunk sems are clean. Applied
        # to a scratch list so a failure leaves the (still-correct,
        # slightly slower) unhoisted layout intact.
        try:
            entry = nc.main_func.blocks[0]
            insts = list(entry.instructions)
            for eng_obj in (nc.sync, nc.scalar):
                loads = load_by_eng[id(eng_obj)]
                for inst in loads:
                    insts.remove(inst)
                drains = [
                    pos
                    for pos, inst in enumerate(insts)
                    if isinstance(inst, mybir.InstDrain)
                    and inst.engine == eng_obj.engine
                ]
                assert len(drains) == 1, drains  # the barrier entry
                idx = drains[0] + 1
                insts[idx:idx] = loads

            entry.instructions[:] = insts
        except Exception:
            return _build_nc(optimize=False)

    nc.finalize()
    return nc


def _run_spmd(x: np.ndarray, **kwargs):
    """x: full [32,3,512,512] f32. Returns BassKernelResults."""
    global _nc_cache
    if _nc_cache is None:
        _nc_cache = _build_nc()
    shards = np.ascontiguousarray(x).reshape(N_CORES, PER_CORE, P, F)
    in_maps = [{"x": shards[i]} for i in range(N_CORES)]
    return run_bass_kernel_spmd(
        _nc_cache, in_maps, core_ids=list(range(N_CORES)), **kwargs
    )


def kernel(input_image: np.ndarray) -> np.ndarray:
    x = np.asarray(input_image, dtype=np.float32)
    res = _run_spmd(x)
    # [8, 128, NCHUNK] -> per-image mins -> mean over 32 images
    partials = np.stack([r["out"] for r in res.results])  # [8, P, NCHUNK]
    col_img = np.asarray(COL_IMG)
    per_image = np.stack(
        [partials[:, :, col_img == b].min(axis=(1, 2)) for b in range(PER_CORE)],
        axis=1,
    )  # [8, PER_CORE]
    return np.asarray(per_image.mean(), dtype=np.float32)


# revision 17
# speedup vs baseline: 1.1548x; 1.1548x over previous
"""Dark channel prior loss on 8 trn2 NeuronCores.

Reference computes: reflect-pad H/W by 7, min over (C, H, W) per image,
mean over batch. Reflect padding only duplicates interior values, so it
cannot change a min — the loss is exactly mean_b(min_chw(x[b])).

Data-parallel: 4 images (12 MiB) per core, streamed as 18 chunks of
[128, .] f32 tapering 2048 -> 1536 -> 1024 columns (scheduled so
VectorE's pipelined per-chunk reduces drain within ~one small chunk of
the stream's end — see the DP note at CHUNK_SIZES).

Loads ride the SP-engine HWDGE ring (nc.sync.dma_start), not SWDGE:
SWDGE keeps its descriptor rings in SBUF partitions whose AXI ports
also serve SDMA engines 7/15, and the resulting fetch contention ran
engine 15 at ~22.5 GB/s vs ~25.7 for the rest — every chunk's 16-inc
completion waited on it, pushing the last reduce ~7 us past the end of
the stream. HWDGE descriptors are RTL-generated (no SBUF ring), the
engines stay uniform, and dispatch skips the ~650 ns/DMA Q7 emission.

Raw bacc kernel (no TileContext): the load DMAs are spliced between
SP's init-barrier entry (so the barrier's gather inc — and DVE's
release — isn't held hostage by the ~0.6-1.5 us/DMA dispatch train)
and its barrier-exit wait (so the stream starts ~0.5 us before Pool
releases the barrier). VectorE min-reduces each chunk to one column of
a [128, 18] partial as its chunk's completion sem hits 16 (one sem per
DMA — a shared cumulative counter is unsound across interleaved
per-engine increments). The partial is DMA'd out on Pool/SWDGE with no
completion wait (the runtime's end-of-program Pool DRAIN blocks on
SWDGE queue completion — measured) while one range-clear resets the
sems for repeat executions. The host finishes min-over-partitions/
chunks and the batch mean (tiny).

History (max-core / mean, us): SWDGE loads 50.5/45.6 (random
~22.4 GB/s straggler SDMA engine on 1-3 cores serializes the tail —
it moves between runs/cores/engines and hits HWDGE too; the kernel
just minimizes the tail it can gate). HWDGE loads, 7x3072+taper:
48.1/44.8 (tail reduces stack serially behind big-chunk reduces).
24x1024 + SP out-DMA: 62.1/58.1 (loads-before-barrier-entry held DVE's
release hostage for the whole 24-DMA dispatch train; out-on-SP put the
out receipt on the critical path). 2048->1024 clean-core-optimal
taper: 49.1/44.3 (clean cores 42.5-43.1, but straggler cores pay small
packets twice). This straggler-aware schedule: 44.0/43.0. Dual-ring
loads (SP+ACT alternating): 55.3/47.0 — ring round-robin adds
turnaround, no bandwidth gain.
"""

import numpy as np

import concourse.bass as bass  # noqa: F401
from concourse import bacc, mybir
from concourse.bass_utils import run_bass_kernel_spmd


def _install_ntff_hook():
    """This image's antenv lacks axon_hooks, so a traced run (trace=True or
    BASS_TRACE=1) would crash inside run_bass_kernel_spmd on the import.
    Synthesize the module around trn_boot's ctypes NTFF hook; degrade
    silently if any piece is missing."""
    import sys
    import types

    if "antenv.axon_hooks" in sys.modules:
        return
    try:
        sys.path.insert(0, "/root/.axon_site")
        from trn_agent_boot.trn_boot import _ntff_profile_via_ctypes

        hook = _ntff_profile_via_ctypes("/opt/axon/libaxon_pjrt.so")
        mod = types.ModuleType("antenv.axon_hooks")
        mod._hook = hook
        mod.get_axon_ntff_profile_hook = lambda: mod._hook
        mod.set_axon_ntff_profile_hook = lambda h: setattr(mod, "_hook", h)
        sys.modules["antenv.axon_hooks"] = mod
    except Exception:
        pass


_install_ntff_hook()

N_CORES = 8
B = 32
PER_CORE = B // N_CORES  # 4 images per core
P = 128
F = 3 * 512 * 512 // P  # 6144 elements per partition per image
TOTAL = PER_CORE * F  # 24576 columns of [128, .] per core

# Chunk column-widths: 3072 -> 512 taper, DP-optimized for the
# STRAGGLER core. The last reduce ends at max_c(land_c +
# suffix_reduce_work_c); DVE reduces at ~1.10 ns/col + 90 ns/op, the
# stream delivers at ~(1.18 + 72/w) ns/col (per-row descriptors are
# w*4 bytes, so narrow chunks pay more per-packet overhead). Nearly
# every launch has 1-2 cores where ONE SDMA engine runs ~13-18% slow
# (worse for small packets; engine/core varies per run — seen on SWDGE
# and HWDGE alike), and the harness takes max over cores, so the DP
# objective dilates stream time by (1.12 + 80/w) per chunk: big early
# chunks minimize the straggler's per-packet overhead, the small tail
# keeps the post-stream reduce drain short on every core. Boundaries
# stay inside images (F=6144 aligned), so each partial column belongs
# to one image.
CHUNK_SIZES = [3072] * 4 + [2048] * 3 + [1536] * 2 + [1024, 768, 768, 512]
assert sum(CHUNK_SIZES) == TOTAL
CHUNK_STARTS = [sum(CHUNK_SIZES[:i]) for i in range(len(CHUNK_SIZES))]
for _s, _w in zip(CHUNK_STARTS, CHUNK_SIZES):
    assert _s // F == (_s + _w - 1) // F
NCHUNK = len(CHUNK_SIZES)
COL_IMG = [s // F for s in CHUNK_STARTS]  # partial column -> image index

_nc_cache = None


def _build_nc(optimize: bool = True):
    nc = bacc.Bacc(trn_type="TRN2", debug=False, num_devices=N_CORES)
    x = nc.dram_tensor("x", [PER_CORE, P, F], mybir.dt.float32, kind="ExternalInput")
    out = nc.dram_tensor("out", [P, NCHUNK], mybir.dt.float32, kind="ExternalOutput")
    x_ap = x.ap()
    out_ap = out.ap()

    # One completion sem per chunk DMA, waited to exactly 16 (one inc per
    # SDMA engine). A single cumulative counter would be unsound: engine k
    # of a LATER chunk can increment before engine j of chunk c finishes,
    # satisfying a >=16*(c+1) wait while chunk c's rows are still in
    # flight (observed as sporadic wrong partials).
    chunk_sems = [nc.alloc_semaphore(f"dma_done_{c}") for c in range(NCHUNK)]
    red_sem = nc.alloc_semaphore("red_done")
    out_sem = nc.alloc_semaphore("out_done")
    buf = nc.alloc_sbuf_tensor("buf", [P, TOTAL], mybir.dt.float32)
    partial = nc.alloc_sbuf_tensor("partial", [P, NCHUNK], mybir.dt.float32)

    # All loads ride SP's single HWDGE ring (qSPDynamicHW): splitting
    # chunks across SP+ACT rings was measured WORSE (mean +3.9 us,
    # stragglers amplified to 55 us) — the SDMA engines' packet-granular
    # round-robin between two fed rings adds turnaround without any
    # bandwidth gain, and chunks land in pairs, delaying the reduce
    # pipeline.
    load_insts = []
    for c, (s, w) in enumerate(zip(CHUNK_STARTS, CHUNK_SIZES)):
        b, off = s // F, s % F  # chunks never straddle an image boundary
        bi = nc.sync.dma_start(
            buf.ap()[:, s : s + w], x_ap[b][:, off : off + w]
        ).then_inc(chunk_sems[c], 16)
        load_insts.append(bi.ins)
    for c, (s, w) in enumerate(zip(CHUNK_STARTS, CHUNK_SIZES)):
        red = nc.vector.tensor_reduce(
            out=partial.ap()[:, c : c + 1],
            in_=buf.ap()[:, s : s + w],
            axis=mybir.AxisListType.X,
            op=mybir.AluOpType.min,
        )._wait_ge(chunk_sems[c], 16)
        if c == NCHUNK - 1:
            red.then_inc(red_sem)
    out_bi = nc.gpsimd.dma_start(out_ap[:], partial.ap())._wait_ge(
        red_sem, 1
    ).then_inc(out_sem, 16)
    # The out-DMA stays on Pool/SWDGE: routing it via SP's HWDGE ring and
    # gating the clear on out_sem>=16 was measured +1.1 us — it saves the
    # ~0.67 us Q7 emission but puts the out's HBM write receipt on the
    # critical path before the done protocol. Here the range-clear (which
    # resets kernel sems so a repeat execution of the same NEFF starts
    # clean) runs right after the out-DMA's Q7 emission and overlaps its
    # flight; chunk/red sems are final-valued once red_sem fired. Nothing
    # waits on out_sem (the DMA lowering just needs an update target):
    # the runtime's end-of-program Pool DRAIN blocks until the SWDGE
    # queue has fully completed (measured), which is what guarantees the
    # output landed before the NEFF execution retires.
    assert out_sem.num == chunk_sems[0].num + NCHUNK + 1
    nc.gpsimd.sem_clear(range(chunk_sems[0].num, out_sem.num + 1))

    if optimize:
        # Hoist the load DMAs to right after SP's init-barrier ENTRY (its
        # InstDrain, which also incs the barrier gather sem) and before
        # its barrier-exit wait. Placement matters on both sides:
        #  - After the entry inc, NOT before it: SP dispatches each
        #    DMACopy in ~0.6-1.5 us (qSPDynamicHW backpressure included),
        #    so loads placed before the barrier entry hold the gather inc
        #    — and with it DVE's release — hostage for the whole dispatch
        #    train (measured +13 us: every reduce then serialized after
        #    the stream).
        #  - Before the exit wait, NOT after it: the barrier release only
        #    comes once Pool's memsets/drain finish (~6 us), while SP is
        #    ready at ~5.4 us — dispatching the first load immediately
        #    starts the HBM stream ~0.5 us earlier.
        # The drain itself stays real: SP has no outstanding DMAs at the
        # barrier (the loads now follow it), so it completes in ns. The
        # loads still execute after the NRT pseudo-barrier that fences
        # the init-time sem_clear, so the chunk sems are clean. Applied
        # to a scratch list so a failure leaves the (still-correct,
        # slightly slower) unhoisted layout intact.
        try:
            entry = nc.main_func.blocks[0]
            insts = list(entry.instructions)
            sp = nc.sync.engine
            for inst in load_insts:
                insts.remove(inst)
            sp_drains = [
                pos
                for pos, inst in enumerate(insts)
                if isinstance(inst, mybir.InstDrain) and inst.engine == sp
            ]
            assert len(sp_drains) == 1, sp_drains  # the barrier entry
            idx = sp_drains[0] + 1
            insts[idx:idx] = load_insts

            entry.instructions[:] = insts
        except Exception:
            return _build_nc(optimize=False)

    nc.finalize()
    return nc


def _run_spmd(x: np.ndarray, **kwargs):
    """x: full [32,3,512,512] f32. Returns BassKernelResults."""
    global _nc_cache
    if _nc_cache is None:
        _nc_cache = _build_nc()
    shards = np.ascontiguousarray(x).reshape(N_CORES, PER_CORE, P, F)
    in_maps = [{"x": shards[i]} for i in range(N_CORES)]
    return run_bass_kernel_spmd(
        _nc_cache, in_maps, core_ids=list(range(N_CORES)), **kwargs
    )


def kernel(input_image: np.ndarray) -> np.ndarray:
    x = np.asarray(input_image, dtype=np.float32)
    res = _run_spmd(x)
    # [8, 128, NCHUNK] -> per-image mins -> mean over 32 images
    partials = np.stack([r["out"] for r in res.results])  # [8, P, NCHUNK]
    col_img = np.asarray(COL_IMG)
    per_image = np.stack(
        [partials[:, :, col_img == b].min(axis=(1, 2)) for b in range(PER_CORE)],
        axis=1,
    )  # [8, PER_CORE]
    return np.asarray(per_image.mean(), dtype=np.float32)
